# revision 45
# baseline (speedup 1.0000x reference)
"""Trainium2 Bass kernel for a GCN critic network (2x GCNConv + host readout).

Strategy: graphs are 32 nodes with no cross-graph edges, so the GCN
scatter-add is a dense 32x32 normalized-adjacency matmul per graph. Four
graphs pack into one 128x128 block-diagonal stationary operand for the
TensorEngine. Data-parallel over 8 cores (512 graphs / 128 blocks per core).

Pipeline (bf16 matmuls, fp32 PSUM accumulation), processed per quarter
(32 blocks) so compute overlaps the DMA tail:
  P1: A0t[f,d] per block  (lhsT=x_b [s,f], rhs=Ablk_b [s,d])
  P2: h1t = relu(W1^T @ A0t)  two weight-stationary passes (w1a then w1b)
  P3: g_b = h1_b @ W2         (node-major, h1t halves as stationary)
  P4: host-only agg: lhsT=Ahost_b [s, 4*16] (13 hosts + 3 pad per graph),
      two blocks share PSUM columns via col-group tile_position
  RO: relu on ACT, mask-mult on the otherwise-idle gpsimd, channel
      reduce on DVE (last quarter: short DVE-only chains to cut the
      tail); final G16 matmul reduces 16-slot partition groups.

All PSUM lives in one pool of 2-bank [128,1024] tiles (bufs=4 = all 8
banks): evictions are 1024 elements wide, halving PSUM->SBUF op count,
and alternate DVE/ACT on a fixed schedule that keeps both engines evenly
loaded (~9us/quarter each, just under the PE's ~9.5).

DMA: 20 sub-DMAs (consts+slab0 on sync, rest on the gpsimd queue,
Ahost quarters interleaved early) so the first matmul input lands ~11us
in; 8 warmup matmuls on a zeroed scratch tile plus 4-matmul fillers
between quarter-0 P1 tiles keep the PE HAM clock at 2.4GHz through the
DMA-chasing phase (a single >3.4us PE idle window would re-throttle the
clock to 1.2GHz).
"""

from contextlib import ExitStack

import numpy as np
import ml_dtypes

NG = 4096
NPG = 32
NH = 13
IN_DIM = 128
H1 = 256
H2 = 64
NCORES = 8
GPC = NG // NCORES          # graphs per core = 512
BPC = GPC // 4              # blocks per core = 128 (4 graphs / block)
NPC = GPC * NPG             # nodes per core = 16384

# mega layout (elements per partition, bf16)
OFF_W1 = 0                  # [128, 256]
OFF_W2 = 256                # [128, 128] = [w2a | w2b]
OFF_MASK = 384              # [128, 1024] mask16 replicated
OFF_G16 = 1408              # [128, 8]
OFF_MAIN = 1536             # 16 sub-slabs of [A 8x128 | x 8x128] = 2048 each
SLAB = 2048
OFF_AH = OFF_MAIN + 16 * SLAB   # 34304: Ahost, 128 blocks x 64
MEGA_W = OFF_AH + BPC * 64      # 42496

BF16 = ml_dtypes.bfloat16

_CACHE = {}


def _a_off(b):
    return OFF_MAIN + SLAB * (b // 8) + 128 * (b % 8)


def _x_off(b):
    return OFF_MAIN + SLAB * (b // 8) + 1024 + 128 * (b % 8)


def _build_bass():
    import concourse.bass as bass
    import concourse.mybir as mybir
    import concourse.tile as tile
    from concourse import bacc
    from concourse.bass import ds

    bf = mybir.dt.bfloat16
    f32 = mybir.dt.float32
    AF = mybir.ActivationFunctionType
    ALU = mybir.AluOpType

    nc = bacc.Bacc("TRN2", target_bir_lowering=False, debug=False)

    megadev = nc.declare_dram_parameter("megadev", [128, MEGA_W], bf, isOutput=False)
    outdev = nc.declare_dram_parameter("outdev", [8, 64], f32, isOutput=True)

    with tile.TileContext(nc) as tc:
        ctx = ExitStack()
        ppool = ctx.enter_context(tc.tile_pool(name="persist", bufs=1))
        mega = ppool.tile([128, MEGA_W], bf, name="mega", tag="mega")
        v_all = ppool.tile([128, 64], f32, name="v_all", tag="v_all")
        vb = ppool.tile([128, 64], bf, name="vb", tag="vb")
        out_sb = ppool.tile([8, 64], f32, name="out_sb", tag="out_sb")
        warm = ppool.tile([128, 512], bf, name="warm", tag="warm")

        w1 = [mega[:, ds(OFF_W1, 128)], mega[:, ds(OFF_W1 + 128, 128)]]
        w2 = [mega[:, ds(OFF_W2, H2)], mega[:, ds(OFF_W2 + H2, H2)]]
        mask512 = mega[:, ds(OFF_MASK, 512)]
        g16 = mega[:, ds(OFF_G16, 8)]

        evict_n = [0]

        with tc.tile_pool(name="a0p", bufs=2) as a0pool, \
             tc.tile_pool(name="h1p", bufs=2) as h1pool, \
             tc.tile_pool(name="gp", bufs=2) as gpool, \
             tc.tile_pool(name="ev", bufs=2) as evpool, \
             tc.tile_pool(name="psb", bufs=4, space="PSUM") as psbp:

            def ptile():
                return psbp.tile([128, 1024], f32, tag="psb", bufs=4, name="ps")

            def evict(dst, src, relu, eng):
                # explicit engine schedule: DVE:ACT ~ 6:8 per quarter since
                # DVE also owns the readout reduce
                if eng == "D":
                    if relu:
                        nc.vector.tensor_scalar_max(dst, src, 0.0)
                    else:
                        nc.vector.tensor_scalar_add(dst, src, 0.0)
                else:
                    if relu:
                        nc.scalar.activation(dst, src, AF.Relu)
                    else:
                        nc.scalar.copy(dst, src)

            # PE warmup during DMA prologue: matmuls on a zeroed scratch tile
            nc.vector.memset(warm[:], 0.0)
            pw = ptile()

            def warmmm(n, width=256):
                for _ in range(n):
                    nc.tensor.matmul(pw[:, ds(0, width)],
                                     lhsT=warm[:, ds(0, 128)],
                                     rhs=warm[:, ds(0, width)],
                                     start=True, stop=True)

            warmmm(10)

            # DMAs: consts + sub-slab 0 first (sync queue), rest on gpsimd
            nc.sync.dma_start(mega[:, ds(0, OFF_MAIN + SLAB)],
                              megadev[:, ds(0, OFF_MAIN + SLAB)])
            order = []
            for s in range(1, 16):
                order.append(OFF_MAIN + s * SLAB)
                if s % 4 == 3:           # after s3, s7, s11: Ahost quarter
                    order.append(OFF_AH + (s // 4) * SLAB)
            order.append(OFF_AH + 3 * SLAB)
            for off in order:
                nc.gpsimd.dma_start(mega[:, ds(off, SLAB)], megadev[:, ds(off, SLAB)])

            for q in range(4):
                a0q = a0pool.tile([128, 4096], bf, tag="a0")
                h1q = [h1pool.tile([128, 4096], bf, tag="h1a", name="h1aq"),
                       h1pool.tile([128, 4096], bf, tag="h1b", name="h1bq")]

                def p1_tile(t, q=q, a0q=a0q):
                    # P1: A0t per block, 8 blocks per 2-bank psum tile
                    ps = ptile()
                    for j in range(8):
                        b = 32 * q + 8 * t + j
                        nc.tensor.matmul(ps[:, ds(128 * j, 128)],
                                         lhsT=mega[:, ds(_x_off(b), 128)],
                                         rhs=mega[:, ds(_a_off(b), 128)],
                                         start=True, stop=True)
                    evict(a0q[:, ds(1024 * t, 1024)], ps[:], False, "DADA"[t])

                def p2_pair(half, k, a0q=a0q, h1q=h1q):
                    # P2: weight-stationary x@W1 on a0 chunk pair k
                    c = 2 * k
                    ps = ptile()
                    nc.tensor.matmul(ps[:, ds(0, 512)], lhsT=w1[half],
                                     rhs=a0q[:, ds(512 * c, 512)],
                                     start=True, stop=True)
                    nc.tensor.matmul(ps[:, ds(512, 512)], lhsT=w1[half],
                                     rhs=a0q[:, ds(512 * (c + 1), 512)],
                                     start=True, stop=True)
                    evict(h1q[half][:, ds(512 * c, 1024)], ps[:], True,
                          "ADADADAA"[4 * half + k])

                if q == 0:
                    # DMA-chase quarter: weave P2 pairs (which depend only
                    # on already-landed P1 tiles) between DMA-gated P1 tiles
                    p1_tile(0); p2_pair(0, 0)
                    p1_tile(1); p2_pair(0, 1)
                    p1_tile(2); p2_pair(1, 0)
                    p1_tile(3); p2_pair(1, 1)
                    p2_pair(0, 2); p2_pair(1, 2)
                    p2_pair(0, 3); p2_pair(1, 3)
                else:
                    for t in range(4):
                        p1_tile(t)
                    for half in range(2):
                        for k in range(4):
                            p2_pair(half, k)

                if q == 3:
                    # first output half: v_all[:, :32] settled since q1
                    hsl = ds(0, 32)
                    nc.vector.tensor_scalar_add(vb[:, hsl], v_all[:, hsl], 0.0)
                    opsA = ptile()
                    nc.tensor.matmul(opsA[ds(0, 8), ds(0, 32)], lhsT=g16,
                                     rhs=vb[:, hsl], start=True, stop=True)
                    nc.scalar.copy(out_sb[:, hsl], opsA[ds(0, 8), ds(0, 32)])

                # ---- P3: g = h1 @ W2, 16 blocks per psum tile ----
                gq = gpool.tile([128, 2048], bf, tag="g8")
                for t in range(2):
                    ps = ptile()
                    for j in range(16):
                        nb = 16 * t + j
                        nsl = ds(128 * nb, 128)
                        osl = ds(64 * j, 64)
                        nc.tensor.matmul(ps[:, osl], lhsT=h1q[0][:, nsl],
                                         rhs=w2[0], start=True, stop=False)
                        nc.tensor.matmul(ps[:, osl], lhsT=h1q[1][:, nsl],
                                         rhs=w2[1], start=False, stop=True)
                    evict(gq[:, ds(1024 * t, 1024)], ps[:], False, "DA"[t])

                # ---- P4 + readout: host rows only, whole quarter in 1 tile ----
                ps = ptile()
                for cp in range(16):
                    for cg in range(2):
                        bq = 2 * cp + cg
                        b = 32 * q + bq
                        nc.tensor.matmul(
                            ps[ds(64 * cg, 64), ds(64 * cp, 64)],
                            lhsT=mega[:, ds(OFF_AH + 64 * b, 64)],
                            rhs=gq[:, ds(64 * bq, 64)],
                            start=True, stop=True,
                            tile_position=(0, 64 * cg),
                        )
                if q < 3:
                    # relu on ACT (frees psum), mask-mult on idle gpsimd,
                    # reduce on DVE - all off the PE critical path
                    h2r = evpool.tile([128, 1024], bf, tag="h2r")
                    nc.scalar.activation(h2r[:], ps[:], AF.Relu)
                    h2m = evpool.tile([128, 1024], bf, tag="h2m")
                    nc.gpsimd.tensor_mul(h2m[:], h2r[:],
                                         mega[:, ds(OFF_MASK, 1024)])
                    nc.vector.tensor_reduce(
                        v_all[:, ds(16 * q, 16)],
                        h2m[:].rearrange("p (j c) -> p j c", c=H2),
                        axis=mybir.AxisListType.X,
                        op=ALU.add,
                    )
                else:
                    # last quarter: short DVE chains in 512-halves for a
                    # minimal tail
                    for hh in range(2):
                        h2m = evpool.tile([128, 512], bf, tag="h2m3",
                                          name="h2m3")
                        nc.vector.scalar_tensor_tensor(
                            h2m[:], ps[:, ds(512 * hh, 512)], 0.0,
                            mega[:, ds(OFF_MASK, 512)],
                            op0=ALU.max, op1=ALU.mult,
                        )
                        nc.vector.tensor_reduce(
                            v_all[:, ds(16 * q + 8 * hh, 8)],
                            h2m[:].rearrange("p (j c) -> p j c", c=H2),
                            axis=mybir.AxisListType.X,
                            op=ALU.add,
                        )

            # ---- final per-graph group reduce (second half) ----
            hsl = ds(32, 32)
            nc.vector.tensor_scalar_add(vb[:, hsl], v_all[:, hsl], 0.0)
            ops = ptile()
            nc.tensor.matmul(ops[ds(0, 8), ds(0, 32)], lhsT=g16,
                             rhs=vb[:, hsl], start=True, stop=True)
            nc.scalar.copy(out_sb[:, hsl], ops[ds(0, 8), ds(0, 32)])
            nc.sync.dma_start(outdev[:, :], out_sb[:])

        ctx.close()

    nc.compile()
    return nc


def _prep_inputs(x, ei, host_idx, W1, b1, W2, b2, Wout, bout):
    """Host-side: dense per-graph adjacency, packed layouts, sharding.
    Returns (in_maps, bout_val) or None if structural assumptions fail."""
    x = np.asarray(x); ei = np.asarray(ei); host_idx = np.asarray(host_idx)
    W1 = np.asarray(W1); b1 = np.asarray(b1); W2 = np.asarray(W2)
    b2 = np.asarray(b2); Wout = np.asarray(Wout); bout = np.asarray(bout)

    N = NG * NPG
    src = ei[0].astype(np.int64)
    dst = ei[1].astype(np.int64)
    if (src // NPG != dst // NPG).any():
        return None
    hi = host_idx.reshape(NG, NH)
    if not (hi == (np.arange(NG)[:, None] * NPG + np.arange(NH)[None, :])).all():
        return None
    if b1.any() or b2.any():
        return None

    deg = np.bincount(dst, minlength=N).astype(np.float64) + 1.0
    dinv = 1.0 / np.sqrt(deg)
    A = np.zeros((NG, NPG, NPG), dtype=np.float64)
    g = src // NPG
    np.add.at(A, (g, dst % NPG, src % NPG), dinv[src] * dinv[dst])
    A[:, np.arange(NPG), np.arange(NPG)] += (dinv * dinv).reshape(NG, NPG)
    A32 = A.astype(np.float32)        # A32[g][dst_local, src_local]

    WoutR = Wout[:, 0].reshape(NH, H2).astype(np.float32)
    mask16 = np.zeros((128, 1024), dtype=np.float32)
    for p in range(128):
        if p % 16 < NH:
            mask16[p] = np.tile(WoutR[p % 16], 16)
    g16 = np.zeros((128, 8), dtype=np.float32)
    for p in range(128):
        g16[p, p // 16] = 1.0

    w2re = np.empty((128, 2 * H2), dtype=np.float32)
    w2re[:, :H2] = W2[:128]
    w2re[:, H2:] = W2[128:]

    in_maps = []
    for c in range(NCORES):
        mega = np.zeros((128, MEGA_W), dtype=np.float32)
        mega[:, OFF_W1:OFF_W1 + H1] = W1
        mega[:, OFF_W2:OFF_W2 + 2 * H2] = w2re
        mega[:, OFF_MASK:OFF_MASK + 1024] = mask16
        mega[:, OFF_G16:OFF_G16 + 8] = g16

        xc = x[c * NPC:(c + 1) * NPC].reshape(BPC, 128, IN_DIM)
        xc = np.ascontiguousarray(xc.transpose(1, 0, 2))       # [128, BPC, 128]
        Ac = A32[c * GPC:(c + 1) * GPC].reshape(BPC, 4, NPG, NPG)
        Ablk = np.zeros((BPC, 128, 128), dtype=np.float32)
        for j in range(4):
            # Ablk[b][s, d] = A[g][d_local, s_local]  (transposed within graph)
            Ablk[:, 32 * j:32 * (j + 1), 32 * j:32 * (j + 1)] = \
                Ac[:, j].transpose(0, 2, 1)
        Ablk = np.ascontiguousarray(Ablk.transpose(1, 0, 2))   # [128, BPC, 128]
        for s in range(16):
            bs = slice(8 * s, 8 * (s + 1))
            off = OFF_MAIN + s * SLAB
            mega[:, off:off + 1024] = Ablk[:, bs].reshape(128, 1024)
            mega[:, off + 1024:off + 2048] = xc[:, bs].reshape(128, 1024)

        # Ahost[b][s, 16g+m] = A32[4b+g][m, s%32] if s//32==g and m<13
        ah = np.zeros((128, BPC, 64), dtype=np.float32)
        for gj in range(4):
            rows = slice(32 * gj, 32 * (gj + 1))
            cols = slice(16 * gj, 16 * gj + NH)
            # Ac[:, gj] is [BPC, dst, src]; want [src, BPC, host]
            ah[rows, :, cols] = Ac[:, gj, :NH, :].transpose(2, 0, 1)
        mega[:, OFF_AH:OFF_AH + BPC * 64] = ah.reshape(128, BPC * 64)
        in_maps.append({"megadev": mega.astype(BF16)})

    # output remap: res[i, j] -> graph 4*(32*(j//16) + 2*(j%16) + i//4) + i%4
    i_idx, j_idx = np.meshgrid(np.arange(8), np.arange(64), indexing="ij")
    gmap = (4 * (32 * (j_idx // 16) + 2 * (j_idx % 16) + i_idx // 4) + i_idx % 4)
    return in_maps, float(bout[0]), gmap


def _numpy_fallback(x, ei, host_idx, W1, b1, W2, b2, Wout, bout):
    import jax
    jax.config.update("jax_platforms", "cpu")
    import jax.numpy as jnp

    def gcn_conv(xx, eei, W, b):
        Nn = xx.shape[0]
        loop = jnp.arange(Nn, dtype=eei.dtype)
        s = jnp.concatenate([eei[0], loop])
        d = jnp.concatenate([eei[1], loop])
        deg = jax.ops.segment_sum(jnp.ones(d.shape, dtype=xx.dtype), d, num_segments=Nn)
        dinv = jnp.where(deg > 0, jax.lax.rsqrt(deg), 0.0)
        norm = dinv[s] * dinv[d]
        h = xx @ W
        agg = jax.ops.segment_sum(h[s] * norm[:, None], d, num_segments=Nn)
        return agg + b

    h = jax.nn.relu(gcn_conv(jnp.asarray(x), jnp.asarray(ei), jnp.asarray(W1), jnp.asarray(b1)))
    h = jax.nn.relu(gcn_conv(h, jnp.asarray(ei), jnp.asarray(W2), jnp.asarray(b2)))
    host_z = h[jnp.asarray(host_idx)]
    nb = host_idx.shape[0] // NH
    z = host_z.reshape(nb, NH * h.shape[1])
    return np.asarray(z @ jnp.asarray(Wout) + jnp.asarray(bout))


def kernel(**inputs):
    prep = _prep_inputs(**inputs)
    if prep is None:
        return _numpy_fallback(**inputs)
    in_maps, bout_val, gmap = prep

    from concourse.bass_utils import run_bass_kernel_spmd

    if "nc" not in _CACHE:
        _CACHE["nc"] = _build_bass()
    nc = _CACHE["nc"]

    res = run_bass_kernel_spmd(nc, in_maps, core_ids=list(range(NCORES)))
    out = np.empty((NG, 1), dtype=np.float32)
    for c in range(NCORES):
        o = np.asarray(res.results[c]["outdev"])       # [8, 64]
        ofull = np.empty(GPC, dtype=np.float32)
        ofull[gmap.ravel()] = o.ravel()
        out[c * GPC:(c + 1) * GPC, 0] = ofull
    out += bout_val
    return out


# revision 46
# speedup vs baseline: 1.0531x; 1.0531x over previous
"""Trainium2 Bass kernel for a GCN critic network (2x GCNConv + host readout).

Strategy: graphs are 32 nodes with no cross-graph edges, so the GCN
scatter-add is a dense 32x32 normalized-adjacency matmul per graph. Four
graphs pack into one 128x128 block-diagonal stationary operand for the
TensorEngine. Data-parallel over 8 cores (512 graphs / 128 blocks per core).

v2 pipeline (bf16 matmuls, fp32 PSUM accumulation), interleaved per quarter
(32 blocks) so PE overlaps the DMA tail:
  P1: A0t[f,d] per block  (lhsT=x_b [s,f], rhs=Ablk_b [s,d])
  P2: h1t = relu(W1^T @ A0t)  two weight-stationary passes (w1a then w1b)
  P3: g_b = h1_b @ W2         (node-major, h1t halves as stationary)
  P4: host-only agg: lhsT=Ahost_b [s, 4*16] (13 hosts + 3 pad per graph),
      two blocks share a PSUM tile via col-group tile_position
  RO: one scalar_tensor_tensor does relu+mask from PSUM; tensor_reduce
      over channels; final G16 matmul reduces 16-slot groups.

DMA: 21 sub-DMAs (consts+slab0 first on sync, rest on the idle gpsimd
queue) so the first matmul input lands early; ~20 dummy matmuls on a
zeroed scratch tile keep the PE HAM clock warm through the DMA prologue.
"""

from contextlib import ExitStack

import numpy as np
import ml_dtypes

NG = 4096
NPG = 32
NH = 13
IN_DIM = 128
H1 = 256
H2 = 64
NCORES = 8
GPC = NG // NCORES          # graphs per core = 512
BPC = GPC // 4              # blocks per core = 128 (4 graphs / block)
NPC = GPC * NPG             # nodes per core = 16384

# mega layout (elements per partition, bf16)
OFF_W1 = 0                  # [128, 256]
OFF_W2 = 256                # [128, 128] = [w2a | w2b]
OFF_MASK = 384              # [128, 1024] mask16 replicated
OFF_G16 = 1408              # [128, 8]
OFF_MAIN = 1536             # 16 sub-slabs of [A 8x128 | x 8x128] = 2048 each
SLAB = 2048
OFF_AH = OFF_MAIN + 16 * SLAB   # 34304: Ahost, 128 blocks x 64
MEGA_W = OFF_AH + BPC * 64      # 42496

BF16 = ml_dtypes.bfloat16

_CACHE = {}


def _a_off(b):
    return OFF_MAIN + SLAB * (b // 8) + 128 * (b % 8)


def _x_off(b):
    return OFF_MAIN + SLAB * (b // 8) + 1024 + 128 * (b % 8)


def _build_bass():
    import concourse.bass as bass
    import concourse.mybir as mybir
    import concourse.tile as tile
    from concourse import bacc
    from concourse.bass import ds

    bf = mybir.dt.bfloat16
    f32 = mybir.dt.float32
    AF = mybir.ActivationFunctionType
    ALU = mybir.AluOpType

    nc = bacc.Bacc("TRN2", target_bir_lowering=False, debug=False)

    megadev = nc.declare_dram_parameter("megadev", [128, MEGA_W], bf, isOutput=False)
    outdev = nc.declare_dram_parameter("outdev", [8, 64], f32, isOutput=True)

    with tile.TileContext(nc) as tc:
        ctx = ExitStack()
        ppool = ctx.enter_context(tc.tile_pool(name="persist", bufs=1))
        mega = ppool.tile([128, MEGA_W], bf, name="mega", tag="mega")
        v_all = ppool.tile([128, 64], f32, name="v_all", tag="v_all")
        vb = ppool.tile([128, 64], bf, name="vb", tag="vb")
        out_sb = ppool.tile([8, 64], f32, name="out_sb", tag="out_sb")
        warm = ppool.tile([128, 512], bf, name="warm", tag="warm")

        w1 = [mega[:, ds(OFF_W1, 128)], mega[:, ds(OFF_W1 + 128, 128)]]
        w2 = [mega[:, ds(OFF_W2, H2)], mega[:, ds(OFF_W2 + H2, H2)]]
        mask512 = mega[:, ds(OFF_MASK, 512)]
        g16 = mega[:, ds(OFF_G16, 8)]

        evict_n = [0]

        with tc.tile_pool(name="a0p", bufs=2) as a0pool, \
             tc.tile_pool(name="h1p", bufs=2) as h1pool, \
             tc.tile_pool(name="gp", bufs=2) as gpool, \
             tc.tile_pool(name="ev", bufs=2) as evpool, \
             tc.tile_pool(name="psb", bufs=4, space="PSUM") as psbp:

            def ptile():
                return psbp.tile([128, 1024], f32, tag="psb", bufs=4, name="ps")

            def evict(dst, src, relu, eng):
                # explicit engine schedule: DVE:ACT ~ 6:8 per quarter since
                # DVE also owns the readout reduce
                if eng == "D":
                    if relu:
                        nc.vector.tensor_scalar_max(dst, src, 0.0)
                    else:
                        nc.vector.tensor_scalar_add(dst, src, 0.0)
                else:
                    if relu:
                        nc.scalar.activation(dst, src, AF.Relu)
                    else:
                        nc.scalar.copy(dst, src)

            # PE warmup during DMA prologue: matmuls on a zeroed scratch tile
            nc.vector.memset(warm[:], 0.0)
            pw = ptile()

            def warmmm(n, width=256):
                for _ in range(n):
                    nc.tensor.matmul(pw[:, ds(0, width)],
                                     lhsT=warm[:, ds(0, 128)],
                                     rhs=warm[:, ds(0, width)],
                                     start=True, stop=True)

            warmmm(8)

            # DMAs: consts + sub-slab 0 first (sync queue), rest on gpsimd
            nc.sync.dma_start(mega[:, ds(0, OFF_MAIN + SLAB)],
                              megadev[:, ds(0, OFF_MAIN + SLAB)])
            order = []
            for s in range(1, 16):
                order.append(OFF_MAIN + s * SLAB)
                if s % 4 == 3:           # after s3, s7, s11: Ahost quarter
                    order.append(OFF_AH + (s // 4) * SLAB)
            order.append(OFF_AH + 3 * SLAB)
            for off in order:
                nc.gpsimd.dma_start(mega[:, ds(off, SLAB)], megadev[:, ds(off, SLAB)])

            for q in range(4):
                a0q = a0pool.tile([128, 4096], bf, tag="a0")
                # ---- P1: A0t per block, 8 blocks per 2-bank psum tile ----
                for t in range(4):
                    ps = ptile()
                    for j in range(8):
                        b = 32 * q + 8 * t + j
                        nc.tensor.matmul(ps[:, ds(128 * j, 128)],
                                         lhsT=mega[:, ds(_x_off(b), 128)],
                                         rhs=mega[:, ds(_a_off(b), 128)],
                                         start=True, stop=True)
                    evict(a0q[:, ds(1024 * t, 1024)], ps[:], False, "DADA"[t])
                    if q == 0 and t < 3:
                        # fill the DMA-chase gaps so the PE clock stays hot
                        warmmm(4, 512)

                # ---- P2: weight-stationary passes ----
                h1q = [h1pool.tile([128, 4096], bf, tag="h1a", name="h1aq"),
                       h1pool.tile([128, 4096], bf, tag="h1b", name="h1bq")]
                for half in range(2):
                    for c in range(0, 8, 2):
                        ps = ptile()
                        nc.tensor.matmul(ps[:, ds(0, 512)], lhsT=w1[half],
                                         rhs=a0q[:, ds(512 * c, 512)],
                                         start=True, stop=True)
                        nc.tensor.matmul(ps[:, ds(512, 512)], lhsT=w1[half],
                                         rhs=a0q[:, ds(512 * (c + 1), 512)],
                                         start=True, stop=True)
                        evict(h1q[half][:, ds(512 * c, 1024)], ps[:], True,
                              "ADADADAA"[4 * half + c // 2])

                # ---- P3: g = h1 @ W2, 16 blocks per psum tile ----
                gq = gpool.tile([128, 2048], bf, tag="g8")
                for t in range(2):
                    ps = ptile()
                    for j in range(16):
                        nb = 16 * t + j
                        nsl = ds(128 * nb, 128)
                        osl = ds(64 * j, 64)
                        nc.tensor.matmul(ps[:, osl], lhsT=h1q[0][:, nsl],
                                         rhs=w2[0], start=True, stop=False)
                        nc.tensor.matmul(ps[:, osl], lhsT=h1q[1][:, nsl],
                                         rhs=w2[1], start=False, stop=True)
                    evict(gq[:, ds(1024 * t, 1024)], ps[:], False, "DA"[t])

                # ---- P4 + readout: host rows only, whole quarter in 1 tile ----
                ps = ptile()
                for cp in range(16):
                    for cg in range(2):
                        bq = 2 * cp + cg
                        b = 32 * q + bq
                        nc.tensor.matmul(
                            ps[ds(64 * cg, 64), ds(64 * cp, 64)],
                            lhsT=mega[:, ds(OFF_AH + 64 * b, 64)],
                            rhs=gq[:, ds(64 * bq, 64)],
                            start=True, stop=True,
                            tile_position=(0, 64 * cg),
                        )
                if q < 3:
                    # relu on ACT (frees psum), mask-mult on idle gpsimd,
                    # reduce on DVE - all off the PE critical path
                    h2r = evpool.tile([128, 1024], bf, tag="h2r")
                    nc.scalar.activation(h2r[:], ps[:], AF.Relu)
                    h2m = evpool.tile([128, 1024], bf, tag="h2m")
                    nc.gpsimd.tensor_mul(h2m[:], h2r[:],
                                         mega[:, ds(OFF_MASK, 1024)])
                    nc.vector.tensor_reduce(
                        v_all[:, ds(16 * q, 16)],
                        h2m[:].rearrange("p (j c) -> p j c", c=H2),
                        axis=mybir.AxisListType.X,
                        op=ALU.add,
                    )
                else:
                    # last quarter: short DVE chains in 512-halves for a
                    # minimal tail
                    for hh in range(2):
                        h2m = evpool.tile([128, 512], bf, tag="h2m3",
                                          name="h2m3")
                        nc.vector.scalar_tensor_tensor(
                            h2m[:], ps[:, ds(512 * hh, 512)], 0.0,
                            mega[:, ds(OFF_MASK, 512)],
                            op0=ALU.max, op1=ALU.mult,
                        )
                        nc.vector.tensor_reduce(
                            v_all[:, ds(16 * q + 8 * hh, 8)],
                            h2m[:].rearrange("p (j c) -> p j c", c=H2),
                            axis=mybir.AxisListType.X,
                            op=ALU.add,
                        )

            # ---- final per-graph group reduce ----
            nc.vector.tensor_scalar_add(vb[:], v_all[:], 0.0)
            ops = ptile()
            nc.tensor.matmul(ops[ds(0, 8), ds(0, 64)], lhsT=g16, rhs=vb[:],
                             start=True, stop=True)
            nc.scalar.copy(out_sb[:], ops[ds(0, 8), ds(0, 64)])
            nc.sync.dma_start(outdev[:, :], out_sb[:])

        ctx.close()

    nc.compile()
    return nc


def _prep_inputs(x, ei, host_idx, W1, b1, W2, b2, Wout, bout):
    """Host-side: dense per-graph adjacency, packed layouts, sharding.
    Returns (in_maps, bout_val) or None if structural assumptions fail."""
    x = np.asarray(x); ei = np.asarray(ei); host_idx = np.asarray(host_idx)
    W1 = np.asarray(W1); b1 = np.asarray(b1); W2 = np.asarray(W2)
    b2 = np.asarray(b2); Wout = np.asarray(Wout); bout = np.asarray(bout)

    N = NG * NPG
    src = ei[0].astype(np.int64)
    dst = ei[1].astype(np.int64)
    if (src // NPG != dst // NPG).any():
        return None
    hi = host_idx.reshape(NG, NH)
    if not (hi == (np.arange(NG)[:, None] * NPG + np.arange(NH)[None, :])).all():
        return None
    if b1.any() or b2.any():
        return None

    deg = np.bincount(dst, minlength=N).astype(np.float64) + 1.0
    dinv = 1.0 / np.sqrt(deg)
    A = np.zeros((NG, NPG, NPG), dtype=np.float64)
    g = src // NPG
    np.add.at(A, (g, dst % NPG, src % NPG), dinv[src] * dinv[dst])
    A[:, np.arange(NPG), np.arange(NPG)] += (dinv * dinv).reshape(NG, NPG)
    A32 = A.astype(np.float32)        # A32[g][dst_local, src_local]

    WoutR = Wout[:, 0].reshape(NH, H2).astype(np.float32)
    mask16 = np.zeros((128, 1024), dtype=np.float32)
    for p in range(128):
        if p % 16 < NH:
            mask16[p] = np.tile(WoutR[p % 16], 16)
    g16 = np.zeros((128, 8), dtype=np.float32)
    for p in range(128):
        g16[p, p // 16] = 1.0

    w2re = np.empty((128, 2 * H2), dtype=np.float32)
    w2re[:, :H2] = W2[:128]
    w2re[:, H2:] = W2[128:]

    in_maps = []
    for c in range(NCORES):
        mega = np.zeros((128, MEGA_W), dtype=np.float32)
        mega[:, OFF_W1:OFF_W1 + H1] = W1
        mega[:, OFF_W2:OFF_W2 + 2 * H2] = w2re
        mega[:, OFF_MASK:OFF_MASK + 1024] = mask16
        mega[:, OFF_G16:OFF_G16 + 8] = g16

        xc = x[c * NPC:(c + 1) * NPC].reshape(BPC, 128, IN_DIM)
        xc = np.ascontiguousarray(xc.transpose(1, 0, 2))       # [128, BPC, 128]
        Ac = A32[c * GPC:(c + 1) * GPC].reshape(BPC, 4, NPG, NPG)
        Ablk = np.zeros((BPC, 128, 128), dtype=np.float32)
        for j in range(4):
            # Ablk[b][s, d] = A[g][d_local, s_local]  (transposed within graph)
            Ablk[:, 32 * j:32 * (j + 1), 32 * j:32 * (j + 1)] = \
                Ac[:, j].transpose(0, 2, 1)
        Ablk = np.ascontiguousarray(Ablk.transpose(1, 0, 2))   # [128, BPC, 128]
        for s in range(16):
            bs = slice(8 * s, 8 * (s + 1))
            off = OFF_MAIN + s * SLAB
            mega[:, off:off + 1024] = Ablk[:, bs].reshape(128, 1024)
            mega[:, off + 1024:off + 2048] = xc[:, bs].reshape(128, 1024)

        # Ahost[b][s, 16g+m] = A32[4b+g][m, s%32] if s//32==g and m<13
        ah = np.zeros((128, BPC, 64), dtype=np.float32)
        for gj in range(4):
            rows = slice(32 * gj, 32 * (gj + 1))
            cols = slice(16 * gj, 16 * gj + NH)
            # Ac[:, gj] is [BPC, dst, src]; want [src, BPC, host]
            ah[rows, :, cols] = Ac[:, gj, :NH, :].transpose(2, 0, 1)
        mega[:, OFF_AH:OFF_AH + BPC * 64] = ah.reshape(128, BPC * 64)
        in_maps.append({"megadev": mega.astype(BF16)})

    # output remap: res[i, j] -> graph 4*(32*(j//16) + 2*(j%16) + i//4) + i%4
    i_idx, j_idx = np.meshgrid(np.arange(8), np.arange(64), indexing="ij")
    gmap = (4 * (32 * (j_idx // 16) + 2 * (j_idx % 16) + i_idx // 4) + i_idx % 4)
    return in_maps, float(bout[0]), gmap


def _numpy_fallback(x, ei, host_idx, W1, b1, W2, b2, Wout, bout):
    import jax
    jax.config.update("jax_platforms", "cpu")
    import jax.numpy as jnp

    def gcn_conv(xx, eei, W, b):
        Nn = xx.shape[0]
        loop = jnp.arange(Nn, dtype=eei.dtype)
        s = jnp.concatenate([eei[0], loop])
        d = jnp.concatenate([eei[1], loop])
        deg = jax.ops.segment_sum(jnp.ones(d.shape, dtype=xx.dtype), d, num_segments=Nn)
        dinv = jnp.where(deg > 0, jax.lax.rsqrt(deg), 0.0)
        norm = dinv[s] * dinv[d]
        h = xx @ W
        agg = jax.ops.segment_sum(h[s] * norm[:, None], d, num_segments=Nn)
        return agg + b

    h = jax.nn.relu(gcn_conv(jnp.asarray(x), jnp.asarray(ei), jnp.asarray(W1), jnp.asarray(b1)))
    h = jax.nn.relu(gcn_conv(h, jnp.asarray(ei), jnp.asarray(W2), jnp.asarray(b2)))
    host_z = h[jnp.asarray(host_idx)]
    nb = host_idx.shape[0] // NH
    z = host_z.reshape(nb, NH * h.shape[1])
    return np.asarray(z @ jnp.asarray(Wout) + jnp.asarray(bout))


def kernel(**inputs):
    prep = _prep_inputs(**inputs)
    if prep is None:
        return _numpy_fallback(**inputs)
    in_maps, bout_val, gmap = prep

    from concourse.bass_utils import run_bass_kernel_spmd

    if "nc" not in _CACHE:
        _CACHE["nc"] = _build_bass()
    nc = _CACHE["nc"]

    res = run_bass_kernel_spmd(nc, in_maps, core_ids=list(range(NCORES)))
    out = np.empty((NG, 1), dtype=np.float32)
    for c in range(NCORES):
        o = np.asarray(res.results[c]["outdev"])       # [8, 64]
        ofull = np.empty(GPC, dtype=np.float32)
        ofull[gmap.ravel()] = o.ravel()
        out[c * GPC:(c + 1) * GPC, 0] = ofull
    out += bout_val
    return out
